# revision 1
# baseline (speedup 1.0000x reference)
"""Multi-head attention kernel for 8 TRN2 NeuronCores.

Reference: out = einsum('dha,blha->bld', O, softmax(q k^T) v) with
q/k/v = einsum('dha,bld->blha', W, x);  B=4, L=2048, D=1024, H=16, A=64.

Sharding: core c handles batch b = c//2 and head-group hg = c%2 (8 heads,
data parallel on B x tensor parallel on heads). Each core computes a partial
output [L, D] summed over its 8 heads; the host adds the two head-group
partials per batch.

Per-core layout (all "T" = transposed so contractions sit on SBUF partitions):
  phase 1: kT/vT then qT = W^T @ xT via float16 matmuls (x and W shipped as
           fp16 from the host: same accuracy as fp32r here since bf16 scores
           dominate the error, but FWL-fast weight loads and half the stream
           bytes); head pairs packed to M=128, one merged k+v pass over the
           xkv stream; q last so the scores pipeline starts early.
  phase 2: v PE-transposed to natural [Lk, A] bf16 with a ones column
           (softmax denominators come free in the ctx matmul), overlapping
           the first pairs' scores. Per head pair: scoresT[lk,lq] = kT^T qT
           (bf16, K=64); exp on ACT psum->sbuf bf16, [128,1024] tiles (no max
           subtraction: |scores| < ~60 so fp32 exp is safe); ctx_aug[65,lq]
           accumulates v_aug^T @ expT over 16 lk chunks; batched normalize
           (one [4,512] reciprocal per pair + DRAM-bounce partition broadcast
           + DVE multiply) -> ctxT pair tile [128, lq] (odd head placed via
           SBUF->SBUF DMA); output projection K=128 over pair tiles, heads
           summed in PSUM -> DMA fp32 out. Strip-0 output projection is
           interleaved between strip-1 pairs to spread PSUM slot pressure.

Measured on TRN2 (neuron-profile): ~485 us exec (485-515 across thermal
states), rel err 8.0e-3.
"""

import sys

sys.path.insert(0, "/opt/trn_rl_repo")

from contextlib import ExitStack

import numpy as np
import ml_dtypes

import concourse.bass as bass  # noqa: F401
import concourse.tile as tile
from concourse import bacc, mybir
from concourse.bass_utils import run_bass_kernel_spmd
from concourse.masks import make_identity

B, L, D, H, A = 4, 2048, 1024, 16, 64
HC = 8          # heads per core
NP = HC // 2    # head pairs per core
DC = D // 128   # d chunks
LC = L // 128   # l chunks

f32 = mybir.dt.float32
bf16 = mybir.dt.bfloat16
f32r = mybir.dt.float32r
f16 = mybir.dt.float16
ExpF = mybir.ActivationFunctionType.Exp


def build_graph():
    nc = bacc.Bacc("TRN2", target_bir_lowering=False, debug=False, num_devices=8)
    xqT_e = nc.dram_tensor("xqT", [D, L], f16, kind="ExternalInput").ap()
    xkvT_e = nc.dram_tensor("xkvT", [D, L], f16, kind="ExternalInput").ap()
    Qw_e = nc.dram_tensor("Qw", [D, HC * A], f16, kind="ExternalInput").ap()
    Kw_e = nc.dram_tensor("Kw", [D, HC * A], f16, kind="ExternalInput").ap()
    Vw_e = nc.dram_tensor("Vw", [D, HC * A], f16, kind="ExternalInput").ap()
    OwT_e = nc.dram_tensor("OwT", [HC * A, D], bf16, kind="ExternalInput").ap()
    out_e = nc.dram_tensor("out", [L, D], f32, kind="ExternalOutput").ap()

    with tile.TileContext(nc) as tc, ExitStack() as ctx:
        pers = ctx.enter_context(tc.tile_pool(name="pers", bufs=1))
        qT = [pers.tile([128, L], bf16, tag=f"qT{p}", name=f"qT{p}") for p in range(NP)]
        kT = [pers.tile([128, L], bf16, tag=f"kT{p}", name=f"kT{p}") for p in range(NP)]
        # v_aug[h]: [lk chunk part, chunk, 0:64 v | 64 ones | 65 pad]
        vaug = [
            pers.tile([128, LC, 66], bf16, tag=f"vaug{h}", name=f"vaug{h}")
            for h in range(HC)
        ]
        ident = pers.tile([128, 128], bf16, tag="ident", name="ident")
        make_identity(nc, ident[:])
        # warm the ACT exp table during the ramp so the first real exp
        # doesn't pay the ~2.7us table load
        warm = pers.tile([1, 16], f32, tag="warm", name="warm")
        nc.scalar.activation(warm[:], ident[0:1, 0:16], ExpF)
        for h in range(HC):
            nc.vector.memset(vaug[h][:, :, 64:65], 1.0)

        # ---------------- phase 1: projections ----------------
        vtp = ctx.enter_context(tc.tile_pool(name="vtp", bufs=1))
        with tc.tile_pool(name="wp", bufs=1) as wp, \
             tc.tile_pool(name="xin", bufs=4) as xp, \
             tc.tile_pool(name="pp1", bufs=8, space="PSUM") as pp1:
            HA = HC * A
            Qch = [wp.tile([128, 4, HA], f16, tag=f"Qch{i}", name=f"Qch{i}") for i in range(2)]
            Kch = [wp.tile([128, 4, HA], f16, tag=f"Kch{i}", name=f"Kch{i}") for i in range(2)]
            Vch = [wp.tile([128, 4, HA], f16, tag=f"Vch{i}", name=f"Vch{i}") for i in range(2)]

            def w_half_ap(w_e, i):
                # [128 part, 4 d-chunks, HA cols] gather of W[(4i+j)*128+p, c]
                return bass.AP(
                    tensor=w_e.tensor,
                    offset=w_e.offset + i * 4 * 128 * HA,
                    ap=[[HA, 128], [128 * HA, 4], [1, HA]],
                )

            def Kc(d):
                return Kch[d // 4][:, d % 4, :]

            def Vc(d):
                return Vch[d // 4][:, d % 4, :]

            def Qc(d):
                return Qch[d // 4][:, d % 4, :]

            # first halves up front; second halves ride after the first x
            # chunks so the sync queue reaches the x stream in ~2 issues
            nc.sync.dma_start(out=Kch[0][:], in_=w_half_ap(Kw_e, 0))
            nc.sync.dma_start(out=Vch[0][:], in_=w_half_ap(Vw_e, 0))
            vT = [vtp.tile([128, L], bf16, tag=f"vT{p}", name=f"vT{p}") for p in range(NP)]

            # one pass per projection; weights stationary reused across the
            # two 512-wide lq tiles of each half, 8 psum accumulators live
            def proj_pass(which, x_e, Wc, emit_out):
                for lqh in range(2):
                    ps = [
                        [
                            pp1.tile([128, 512], f32, tag="qk", bufs=8,
                                     name=f"ps_{which}_{lqh}_{p}_{j}")
                            for j in range(2)
                        ]
                        for p in range(NP)
                    ]
                    for d in range(DC):
                        xt = xp.tile([128, 1024], f16, tag="x", bufs=6,
                                     name=f"x_{which}_{lqh}_{d}")
                        lo = lqh * 1024
                        nc.sync.dma_start(
                            out=xt[:],
                            in_=x_e[d * 128:(d + 1) * 128, lo:lo + 1024])
                        for p in range(NP):
                            for j in range(2):
                                nc.tensor.matmul(
                                    ps[p][j][:],
                                    lhsT=Wc(d)[:, p * 128:(p + 1) * 128],
                                    rhs=xt[:, j * 512:(j + 1) * 512],
                                    start=(d == 0), stop=(d == DC - 1))
                    for p in range(NP):
                        for j in range(2):
                            emit_out(p, lqh * 2 + j, ps[p][j])

            # merged k+v pass: one xkv stream feeds both projections
            # (8 psum accumulators live: 4 k + 4 v)
            for lq in range(4):
                pk = [pp1.tile([128, 512], f32, tag="qk", bufs=8,
                               name=f"pk{lq}_{p}") for p in range(NP)]
                pv = [pp1.tile([128, 512], f32, tag="qk", bufs=8,
                               name=f"pv{lq}_{p}") for p in range(NP)]
                for d in range(DC):
                    if lq == 0 and d == 2:
                        nc.sync.dma_start(out=Kch[1][:], in_=w_half_ap(Kw_e, 1))
                        nc.sync.dma_start(out=Vch[1][:], in_=w_half_ap(Vw_e, 1))
                    xt = xp.tile([128, 512], f16, tag="xkv", bufs=8,
                                 name=f"xkv_{lq}_{d}")
                    nc.sync.dma_start(
                        out=xt[:],
                        in_=xkvT_e[d * 128:(d + 1) * 128, lq * 512:(lq + 1) * 512])
                    for p in range(NP):
                        nc.tensor.matmul(
                            pk[p][:], lhsT=Kc(d)[:, p * 128:(p + 1) * 128],
                            rhs=xt[:], start=(d == 0), stop=(d == DC - 1))
                    for p in range(NP):
                        nc.tensor.matmul(
                            pv[p][:], lhsT=Vc(d)[:, p * 128:(p + 1) * 128],
                            rhs=xt[:], start=(d == 0), stop=(d == DC - 1))
                for p in range(NP):
                    nc.vector.tensor_copy(kT[p][:, lq * 512:(lq + 1) * 512], pk[p][:])
                    nc.vector.tensor_copy(vT[p][:, lq * 512:(lq + 1) * 512], pv[p][:])

            nc.sync.dma_start(out=Qch[0][:], in_=w_half_ap(Qw_e, 0))
            nc.sync.dma_start(out=Qch[1][:], in_=w_half_ap(Qw_e, 1))
            proj_pass("q", xqT_e, Qc,
                      lambda p, lq, pst: nc.vector.tensor_copy(
                          qT[p][:, lq * 512:(lq + 1) * 512], pst[:]))

        # ---------------- phase 2: attention + output projection ----------------
        # (the v transposes live here so the scores/exp pipeline for pair 0 can
        # start as soon as the q pass drains, overlapping the transposes)
        with tc.tile_pool(name="owp", bufs=1) as owp, \
             tc.tile_pool(name="p2p", bufs=1) as p2p, \
             tc.tile_pool(name="drp", bufs=4, space="DRAM") as drp, \
             tc.tile_pool(name="expp", bufs=10) as ep, \
             tc.tile_pool(name="pp2", bufs=1, space="PSUM") as pp2:
            # v transposes: vT [2h*a, lk] -> v natural [lk, a] per head
            for p in range(NP):
                for c in range(LC):
                    pt = pp2.tile([128, 128], bf16, tag="c", bufs=4, name=f"pt{p}_{c}")
                    nc.tensor.transpose(pt[:], vT[p][:, c * 128:(c + 1) * 128], ident[:])
                    nc.vector.tensor_copy(vaug[2 * p][:, c, 0:64], pt[:, 0:64])
                    nc.vector.tensor_copy(vaug[2 * p + 1][:, c, 0:64], pt[:, 64:128])

            # O weights pair-stacked: chunk c rows = (head 2c | head 2c+1) x a
            ow = [owp.tile([128, D], bf16, tag=f"ow{c}", name=f"ow{c}") for c in range(NP)]
            for c in range(NP):
                nc.sync.dma_start(out=ow[c][:], in_=OwT_e[c * 128:(c + 1) * 128, :])

            def emit_outproj(strip, ctx_tiles, groups):
                for lqs in groups:
                    for dt_ in range(2):
                        po = pp2.tile([128, 512], f32, tag="c", bufs=4,
                                      name=f"po{strip}_{lqs}_{dt_}")
                        for p in range(NP):
                            nc.tensor.matmul(
                                po[:],
                                lhsT=ctx_tiles[p][:, lqs * 128:(lqs + 1) * 128],
                                rhs=ow[p][:, dt_ * 512:(dt_ + 1) * 512],
                                start=(p == 0), stop=(p == NP - 1))
                        ost = p2p.tile([128, 512], f32, tag="ost", bufs=3,
                                       name=f"ost{strip}_{lqs}_{dt_}")
                        nc.vector.tensor_copy(ost[:], po[:])
                        row = strip * 1024 + lqs * 128
                        nc.sync.dma_start(
                            out=out_e[row:row + 128, dt_ * 512:(dt_ + 1) * 512],
                            in_=ost[:])

            strip_ctx = {}
            for strip in range(2):
                ctxp = []
                for p in range(NP):
                    ctp = p2p.tile([128, 1024], bf16, tag="ctxT", bufs=10,
                                   name=f"ctp{strip}_{p}")
                    pcs = [
                        [
                            pp2.tile([65, 512], f32, tag="c", bufs=4,
                                     name=f"pc{strip}_{p}_{h2}_{s}")
                            for s in range(2)
                        ]
                        for h2 in range(2)
                    ]
                    for c in range(LC):
                        sts = [
                            pp2.tile([128, 1024], f32, tag="s", bufs=2,
                                     name=f"st{strip}_{p}_{h2}_{c}")
                            for h2 in range(2)
                        ]
                        # h2-outer: consecutive matmuls reuse the stationary k
                        # chunk; the next head's LDWEIGHTS (disjoint row group,
                        # bases 0/64) pulls ahead of the in-flight matmuls
                        for h2 in range(2):
                            base = 64 * h2
                            for sub in range(2):
                                lo = strip * 1024 + sub * 512
                                nc.tensor.matmul(
                                    sts[h2][:, sub * 512:(sub + 1) * 512],
                                    lhsT=kT[p][base:base + 64, c * 128:(c + 1) * 128],
                                    rhs=qT[p][base:base + 64, lo:lo + 512],
                                    start=True, stop=True)
                        for h2 in range(2):
                            et = ep.tile([128, 1024], bf16, tag="exp",
                                         name=f"et{strip}_{p}_{h2}_{c}")
                            nc.scalar.activation(et[:], sts[h2][:], ExpF)
                            for sub in range(2):
                                nc.tensor.matmul(
                                    pcs[h2][sub][:],
                                    lhsT=vaug[2 * p + h2][:, c, 0:65],
                                    rhs=et[:, sub * 512:(sub + 1) * 512],
                                    start=(c == 0), stop=(c == LC - 1))
                    # batched normalize: gather the pair's 4 denominator rows
                    # (DVE copies into column blocks of one partition-64 row,
                    # then a reshaping SBUF->SBUF DMA onto 4 partitions), one
                    # [4,512] reciprocal, bounce to DRAM, broadcast rows
                    stage = p2p.tile([65, 2048], f32, tag="dstage", bufs=2,
                                     name=f"stage{strip}_{p}")
                    for h2 in range(2):
                        for sub in range(2):
                            r = 2 * h2 + sub
                            nc.vector.tensor_copy(
                                stage[64:65, r * 512:(r + 1) * 512],
                                pcs[h2][sub][64:65, :])
                    den = p2p.tile([4, 512], f32, tag="den", bufs=2,
                                   name=f"den{strip}_{p}")
                    nc.sync.dma_start(out=den[:], in_=stage[64:65, :])
                    rec = p2p.tile([4, 512], f32, tag="rec", bufs=2,
                                   name=f"rec{strip}_{p}")
                    nc.vector.reciprocal(rec[:], den[:])
                    dr = drp.tile([4, 512], f32, tag="dr", bufs=2,
                                  name=f"dr{strip}_{p}")
                    nc.sync.dma_start(out=dr[:], in_=rec[:])
                    # evacuate the 4 ctx accumulators to SBUF right away so
                    # the PSUM slots free for the next pair's ctx matmuls
                    # (the normalize chain below is ~8us of recip+DMA latency)
                    un = [
                        [
                            p2p.tile([64, 512], bf16, tag="un", bufs=8,
                                     name=f"un{strip}_{p}_{h2}_{s}")
                            for s in range(2)
                        ]
                        for h2 in range(2)
                    ]
                    for h2 in range(2):
                        for sub in range(2):
                            nc.vector.tensor_copy(un[h2][sub][:],
                                                  pcs[h2][sub][0:64, :])
                    for h2 in range(2):
                        cto = None
                        if h2 == 1:
                            cto = p2p.tile([64, 1024], bf16, tag="cto", bufs=3,
                                           name=f"cto{strip}_{p}")
                        for sub in range(2):
                            r = 2 * h2 + sub
                            pbs = p2p.tile([64, 512], f32, tag="bcast", bufs=4,
                                           name=f"pbs{strip}_{p}_{h2}_{sub}")
                            dr_row = dr[r:r + 1, :]
                            dr_bcast = bass.AP(
                                tensor=dr_row.tensor,
                                offset=dr_row.offset,
                                ap=[[0, 64], [1, 512]],
                            )
                            nc.sync.dma_start(out=pbs[:], in_=dr_bcast)
                            dst = (ctp[0:64, sub * 512:(sub + 1) * 512]
                                   if h2 == 0 else
                                   cto[:, sub * 512:(sub + 1) * 512])
                            nc.vector.tensor_mul(
                                dst, un[h2][sub][:], pbs[:])
                        if h2 == 1:
                            # odd head into pair-tile partitions 64..127
                            nc.sync.dma_start(out=ctp[64:128, :], in_=cto[:])
                    ctxp.append(ctp)

                    if strip == 1:
                        emit_outproj(0, strip_ctx[0], [2 * len(ctxp) - 2, 2 * len(ctxp) - 1])
                strip_ctx[strip] = ctxp
            emit_outproj(1, strip_ctx[1], list(range(8)))
    nc.compile()
    return nc


_NC = None


def _get_nc():
    global _NC
    if _NC is None:
        _NC = build_graph()
    return _NC


# test harness can override, e.g. {"trace": True}
RUN_KWARGS: dict = {}
LAST_RESULTS = None


def make_in_maps(xq, xkv, Q, K, V, O):
    xq = np.asarray(xq, np.float32)
    xkv = np.asarray(xkv, np.float32)
    Q = np.asarray(Q, np.float32)
    K = np.asarray(K, np.float32)
    V = np.asarray(V, np.float32)
    O = np.asarray(O, np.float32)
    # cores 2b and 2b+1 share batch b's transposed activations; compute once
    xqT_c = [np.ascontiguousarray(xq[b].T).astype(np.float16) for b in range(B)]
    xkvT_c = [np.ascontiguousarray(xkv[b].T).astype(np.float16) for b in range(B)]
    in_maps = []
    for core in range(8):
        b, hg = divmod(core, 2)
        hs = slice(hg * HC, (hg + 1) * HC)
        in_maps.append({
            "xqT": xqT_c[b],
            "xkvT": xkvT_c[b],
            "Qw": np.ascontiguousarray(Q[:, hs, :].reshape(D, HC * A)).astype(np.float16),
            "Kw": np.ascontiguousarray(K[:, hs, :].reshape(D, HC * A)).astype(np.float16),
            "Vw": np.ascontiguousarray(V[:, hs, :].reshape(D, HC * A)).astype(np.float16),
            "OwT": np.ascontiguousarray(
                O[:, hs, :].reshape(D, HC * A).T).astype(ml_dtypes.bfloat16),
        })
    return in_maps


def kernel(xq, xkv, Q, K, V, O):
    global LAST_RESULTS
    nc = _get_nc()
    in_maps = make_in_maps(xq, xkv, Q, K, V, O)
    res = run_bass_kernel_spmd(nc, in_maps, core_ids=list(range(8)), **RUN_KWARGS)
    LAST_RESULTS = res
    outs = [np.asarray(res.results[c]["out"], np.float32) for c in range(8)]
    return np.stack([outs[2 * b] + outs[2 * b + 1] for b in range(B)], axis=0)



# revision 3
# speedup vs baseline: 1.0470x; 1.0470x over previous
"""Multi-head attention kernel for 8 TRN2 NeuronCores.

Reference: out = einsum('dha,blha->bld', O, softmax(q k^T) v) with
q/k/v = einsum('dha,bld->blha', W, x);  B=4, L=2048, D=1024, H=16, A=64.

Sharding: core c handles batch b = c//2 and head-group hg = c%2 (8 heads,
data parallel on B x tensor parallel on heads). Each core computes a partial
output [L, D] summed over its 8 heads; the host adds the two head-group
partials per batch.

Schedule: the kernel is ACT(exp)-bound -- 8 heads x 2048^2 exps as 256
[128,1024] ACTIVATEs ~= 294us of scalar-engine time.  Everything else is
arranged so the scalar engine never idles:
  - all attention sections (scores -> exp -> ctx -> normalize) are issued
    inside one tc.high_priority block; projections, the natural-layout v
    pass, and the output projection ride at natural priority, so the Tile
    scheduler runs them only in PE slack.  There is no serial projection
    phase and (almost) no output-projection tail.
  - scores for the two heads of a pair run concurrently on the two PE row
    halves (K=64 row tiling via base_partition 0/64), writing the two
    bank-halves of one [128,1024] psum tile; one exp ACTIVATE covers both.
  - v is projected directly to natural [lk, head*64] layout (x chunk
    stationary, V weights moving), in two head-halves so pairs 0/1's v is
    ready early; a ones column makes softmax denominators fall out of the
    ctx matmul.  No PE transposes.
  - per (pair, strip, sub): ctx accumulates in 2 psum banks; normalize =
    reciprocal + DRAM-bounce partition broadcast + DVE multiply, hidden
    under the next section's compute.
  - pair 3 processes strip 1 before strip 0, so outproj(strip 1) overlaps
    pair 3's strip-0 sections and only strip 0's outproj trails the last exp.
PSUM: scores staging 2x2 banks + ctx accumulators 2 + proj/outproj scratch 2.
"""

import sys

sys.path.insert(0, "/opt/trn_rl_repo")

from contextlib import ExitStack

import numpy as np
import ml_dtypes

import concourse.bass as bass  # noqa: F401
import concourse.tile as tile
from concourse import bacc, mybir
from concourse.bass_utils import run_bass_kernel_spmd

B, L, D, H, A = 4, 2048, 1024, 16, 64
HC = 8          # heads per core
NP = HC // 2    # head pairs per core
DC = D // 128   # d chunks
LC = L // 128   # lk chunks
HA = HC * A     # 512
NW = 4          # 512-wide lq windows

f32 = mybir.dt.float32
bf16 = mybir.dt.bfloat16
f16 = mybir.dt.float16
ExpF = mybir.ActivationFunctionType.Exp


def build_graph():
    nc = bacc.Bacc("TRN2", target_bir_lowering=False, debug=False, num_devices=8)
    xqT_e = nc.dram_tensor("xqT", [D, L], f16, kind="ExternalInput").ap()
    xkvT_e = nc.dram_tensor("xkvT", [D, L], f16, kind="ExternalInput").ap()
    Qw_e = nc.dram_tensor("Qw", [D, HA], f16, kind="ExternalInput").ap()
    Kw_e = nc.dram_tensor("Kw", [D, HA], f16, kind="ExternalInput").ap()
    Vw_e = nc.dram_tensor("Vw", [D, HA], f16, kind="ExternalInput").ap()
    OwT_e = nc.dram_tensor("OwT", [HA, D], bf16, kind="ExternalInput").ap()
    out_e = nc.dram_tensor("out", [L, D], f32, kind="ExternalOutput").ap()

    with tile.TileContext(nc) as tc, ExitStack() as ctx:
        pers = ctx.enter_context(tc.tile_pool(name="pers", bufs=1))
        sb = ctx.enter_context(tc.tile_pool(name="sb", bufs=1))
        drp = ctx.enter_context(tc.tile_pool(name="drp", bufs=2, space="DRAM"))
        pp = ctx.enter_context(tc.tile_pool(name="pp", bufs=1, space="PSUM"))

        # ---- resident tiles ----
        xkv_s = pers.tile([128, DC, L], f16, tag="xkv", name="xkv_s")
        Qw_s = pers.tile([128, DC, HA], f16, tag="Qw", name="Qw_s")
        Kw_s = pers.tile([128, DC, HA], f16, tag="Kw", name="Kw_s")
        Vw_s = pers.tile([128, DC, HA], f16, tag="Vw", name="Vw_s")
        ow = [pers.tile([128, D], bf16, tag=f"ow{p}", name=f"ow{p}") for p in range(NP)]
        # vaug[lk part, lk chunk, head, 0:64 v | 64 ones | 65 pad]
        vaug = pers.tile([128, LC, HC, 66], bf16, tag="vaug", name="vaug")
        ctp = [[pers.tile([128, 1024], bf16, tag=f"ctp{s}_{p}", name=f"ctp{s}_{p}")
                for p in range(NP)] for s in range(2)]
        warm_in = pers.tile([1, 16], f32, tag="warm_in", name="warm_in")
        warm = pers.tile([1, 16], f32, tag="warm", name="warm")

        def wap(w_e):
            # [128 part, DC d-chunks, HA cols] gather of W[d*128+p, c]
            return bass.AP(
                tensor=w_e.tensor,
                offset=w_e.offset,
                ap=[[HA, 128], [128 * HA, DC], [1, HA]],
            )

        def xwin_ap(x_e, w):
            return bass.AP(
                tensor=x_e.tensor,
                offset=x_e.offset + w * 512,
                ap=[[L, 128], [128 * L, DC], [1, 512]],
            )

        # ---- input DMAs, in priority order ----
        nc.sync.dma_start(out=Kw_s[:], in_=wap(Kw_e))
        nc.sync.dma_start(out=Qw_s[:], in_=wap(Qw_e))
        for w in range(NW):
            nc.sync.dma_start(out=xkv_s[:, :, w * 512:(w + 1) * 512],
                              in_=xwin_ap(xkvT_e, w))
        nc.sync.dma_start(out=Vw_s[:], in_=wap(Vw_e))
        for p in range(NP):
            nc.sync.dma_start(out=ow[p][:], in_=OwT_e[p * 128:(p + 1) * 128, :])

        # warm the ACT exp table during the ramp
        nc.vector.memset(warm_in[:], 0.0)
        nc.scalar.activation(warm[:], warm_in[:], ExpF)
        nc.vector.memset(vaug[:, :, :, 64:65], 1.0)

        # qT/kT pair tiles: rows 0:64 head even, 64:128 head odd; 2 live pairs
        qTs = [sb.tile([128, L], bf16, tag="qT", bufs=2, name=f"qT{p}")
               for p in range(NP)]
        kTs = [sb.tile([128, L], bf16, tag="kT", bufs=2, name=f"kT{p}")
               for p in range(NP)]

        # ---- slack-priority producers ----
        def proj_window(which, p, w):
            """one 512-lq window of the q/k projection for pair p."""
            if which == "q":
                xt = sb.tile([128, DC, 512], f16, tag="xq", bufs=2,
                             name=f"xq_{p}_{w}")
                nc.sync.dma_start(out=xt[:], in_=xwin_ap(xqT_e, w))
                rhs = lambda d: xt[:, d, :]
                ws, dst = Qw_s, qTs[p]
            else:
                rhs = lambda d: xkv_s[:, d, w * 512:(w + 1) * 512]
                ws, dst = Kw_s, kTs[p]
            pj = pp.tile([128, 512], f32, tag="scr", bufs=2,
                         name=f"pj_{which}_{p}_{w}")
            for d in range(DC):
                nc.tensor.matmul(pj[:], lhsT=ws[:, d, p * 128:(p + 1) * 128],
                                 rhs=rhs(d), start=(d == 0), stop=(d == DC - 1))
            nc.vector.tensor_copy(dst[:, w * 512:(w + 1) * 512], pj[:])

        def vnat_chunk(c, half):
            """natural-layout v for lk chunk c, heads half*4..half*4+3."""
            vn = pp.tile([128, 256], f32, tag="scr", bufs=2, name=f"vn_{c}_{half}")
            for d in range(DC):
                nc.tensor.matmul(
                    vn[:], lhsT=xkv_s[:, d, c * 128:(c + 1) * 128],
                    rhs=Vw_s[:, d, half * 256:(half + 1) * 256],
                    start=(d == 0), stop=(d == DC - 1))
            nc.vector.tensor_copy(vaug[:, c, half * 4:(half + 1) * 4, 0:64], vn[:])

        # ---- high-priority attention pipeline ----
        def normalize(p, s, sub, pcs):
            stg = sb.tile([65, 1024], f32, tag="stg", bufs=2, name=f"stg{p}{s}{sub}")
            un = [sb.tile([64, 512], bf16, tag="un", bufs=4,
                          name=f"un{p}{s}{sub}{h2}") for h2 in range(2)]
            for h2 in range(2):
                nc.vector.tensor_copy(stg[64:65, h2 * 512:(h2 + 1) * 512],
                                      pcs[h2][64:65, :])
            for h2 in range(2):
                nc.vector.tensor_copy(un[h2][:], pcs[h2][0:64, :])
            rec = sb.tile([65, 1024], f32, tag="stg", bufs=2, name=f"rec{p}{s}{sub}")
            nc.vector.reciprocal(rec[64:65, :], stg[64:65, :])
            dr = drp.tile([1, 1024], f32, tag="dr", bufs=2, name=f"dr{p}{s}{sub}")
            nc.sync.dma_start(out=dr[:], in_=rec[64:65, :])
            cto = sb.tile([64, 512], bf16, tag="cto", bufs=2, name=f"cto{p}{s}{sub}")
            for h2 in range(2):
                pbs = sb.tile([64, 512], f32, tag="pbs", bufs=4,
                              name=f"pbs{p}{s}{sub}{h2}")
                bcast = bass.AP(tensor=dr.tensor, offset=dr.offset + h2 * 512,
                                ap=[[0, 64], [1, 512]])
                nc.sync.dma_start(out=pbs[:], in_=bcast)
                dst = (ctp[s][p][0:64, sub * 512:(sub + 1) * 512]
                       if h2 == 0 else cto[:])
                nc.vector.tensor_mul(dst, un[h2][:], pbs[:])
            nc.sync.dma_start(out=ctp[s][p][64:128, sub * 512:(sub + 1) * 512],
                              in_=cto[:])

        def attention(p, s, sub):
            qT, kT = qTs[p], kTs[p]
            w = s * 2 + sub
            pcs = [pp.tile([65, 512], f32, tag="pcs", bufs=2,
                           name=f"pc{p}{s}{sub}{h2}") for h2 in range(2)]
            for c in range(LC):
                sts = pp.tile([128, 1024], f32, tag="sts", bufs=2,
                              name=f"st{p}{s}{sub}{c}")
                for h2 in range(2):
                    nc.tensor.matmul(
                        sts[:, h2 * 512:(h2 + 1) * 512],
                        lhsT=kT[h2 * 64:(h2 + 1) * 64, c * 128:(c + 1) * 128],
                        rhs=qT[h2 * 64:(h2 + 1) * 64, w * 512:(w + 1) * 512],
                        start=True, stop=True)
                et = sb.tile([128, 1024], bf16, tag="exp", bufs=10,
                             name=f"et{p}{s}{sub}{c}")
                nc.scalar.activation(et[:], sts[:], ExpF)
                for h2 in range(2):
                    nc.tensor.matmul(pcs[h2][:],
                                     lhsT=vaug[:, c, 2 * p + h2, 0:65],
                                     rhs=et[:, h2 * 512:(h2 + 1) * 512],
                                     start=(c == 0), stop=(c == LC - 1))
            normalize(p, s, sub, pcs)

        def outproj(s, blocks):
            for b in blocks:
                for dt_ in range(2):
                    po = pp.tile([128, 512], f32, tag="scr", bufs=2,
                                 name=f"po{s}_{b}_{dt_}")
                    for p in range(NP):
                        nc.tensor.matmul(
                            po[:], lhsT=ctp[s][p][:, b * 128:(b + 1) * 128],
                            rhs=ow[p][:, dt_ * 512:(dt_ + 1) * 512],
                            start=(p == 0), stop=(p == NP - 1))
                    ost = sb.tile([128, 512], f32, tag="ost", bufs=3,
                                  name=f"ost{s}_{b}_{dt_}")
                    nc.vector.tensor_copy(ost[:], po[:])
                    row = s * 1024 + b * 128
                    nc.sync.dma_start(
                        out=out_e[row:row + 128, dt_ * 512:(dt_ + 1) * 512],
                        in_=ost[:])

        # ---- issue: slack producers at natural priority, deadline-ordered ----
        proj_window("k", 0, 0)
        proj_window("q", 0, 0)
        proj_window("k", 0, 1)
        for c in range(4):
            vnat_chunk(c, 0)
        proj_window("k", 0, 2)
        for c in range(4, 8):
            vnat_chunk(c, 0)
        proj_window("k", 0, 3)
        for c in range(8, 12):
            vnat_chunk(c, 0)
        proj_window("q", 0, 1)
        for c in range(12, LC):
            vnat_chunk(c, 0)
        proj_window("q", 0, 2)
        proj_window("q", 0, 3)
        for p in range(1, NP):
            for w in range(NW):
                proj_window("k", p, w)
            for w in range(NW):
                proj_window("q", p, w)
            if p == 1:
                for c in range(LC):
                    vnat_chunk(c, 1)

        # ---- attention sections, strictly prioritized ----
        with tc.high_priority(offset=10 ** 6):
            for p in range(NP):
                strips = [0, 1] if p < NP - 1 else [1, 0]
                for s in strips:
                    for sub in range(2):
                        attention(p, s, sub)

        # ---- output projections (readiness-gated slack work) ----
        outproj(1, list(range(8)))
        outproj(0, list(range(8)))
    nc.compile()
    return nc


_NC = None


def _get_nc():
    global _NC
    if _NC is None:
        _NC = build_graph()
    return _NC


# test harness can override, e.g. {"trace": True}
RUN_KWARGS: dict = {}
LAST_RESULTS = None


def make_in_maps(xq, xkv, Q, K, V, O):
    xq = np.asarray(xq, np.float32)
    xkv = np.asarray(xkv, np.float32)
    Q = np.asarray(Q, np.float32)
    K = np.asarray(K, np.float32)
    V = np.asarray(V, np.float32)
    O = np.asarray(O, np.float32)
    # cores 2b and 2b+1 share batch b's transposed activations; compute once
    xqT_c = [np.ascontiguousarray(xq[b].T).astype(np.float16) for b in range(B)]
    xkvT_c = [np.ascontiguousarray(xkv[b].T).astype(np.float16) for b in range(B)]
    in_maps = []
    for core in range(8):
        b, hg = divmod(core, 2)
        hs = slice(hg * HC, (hg + 1) * HC)
        in_maps.append({
            "xqT": xqT_c[b],
            "xkvT": xkvT_c[b],
            "Qw": np.ascontiguousarray(Q[:, hs, :].reshape(D, HA)).astype(np.float16),
            "Kw": np.ascontiguousarray(K[:, hs, :].reshape(D, HA)).astype(np.float16),
            "Vw": np.ascontiguousarray(V[:, hs, :].reshape(D, HA)).astype(np.float16),
            "OwT": np.ascontiguousarray(
                O[:, hs, :].reshape(D, HA).T).astype(ml_dtypes.bfloat16),
        })
    return in_maps


def kernel(xq, xkv, Q, K, V, O):
    global LAST_RESULTS
    nc = _get_nc()
    in_maps = make_in_maps(xq, xkv, Q, K, V, O)
    res = run_bass_kernel_spmd(nc, in_maps, core_ids=list(range(8)), **RUN_KWARGS)
    LAST_RESULTS = res
    outs = [np.asarray(res.results[c]["out"], np.float32) for c in range(8)]
    return np.stack([outs[2 * b] + outs[2 * b + 1] for b in range(B)], axis=0)
